# revision 2
# baseline (speedup 1.0000x reference)
"""Trainium2 Bass kernel: row-wise cosine similarity discriminator.

Computes, for full inputs s, h_rl, h_fk of shape [B=8, N=8192, D=512] f32:
    out = concat(rowdot(l2n(s), l2n(h_rl)), rowdot(l2n(s), l2n(h_fk)), axis=1)
with l2n(x) = x / max(||x||_2, 1e-12), giving out shape [8, 16384] f32.

Sharding: pure data parallel over batch B — core b processes batch b.

Per-core kernel strategy (memory-bound: 48 MiB input / core, HBM floor
~141 us at ~358 GB/s/core). All engine work is kept under that floor by
fusing every row-reduction into the instruction that produces it:
  - DVE  scalar_tensor_tensor(mult, accum_out): s*h_rl dot and h_fk^2
    norm, one [128, 512] tile-job each (~0.7 us)
  - ACT  activation(Square, accum_out): s^2 and h_rl^2 norms (~0.9 us
    per tile-job incl. the 186 ns ACTIVATION_READ_ACCUMULATOR)
  - GPS  tensor_tensor mult + pairwise halve for s*h_fk; DVE does the
    final [128, 8, 256] tensor_reduce (gpsimd cannot reduce free-dim)
  - data streamed in 8 slots of [128, 8, 512] x 3 tensors (2 MiB HWDGE
    DMAs), double-buffered; finals on [128, 64] stats tiles, output
    transposed on the idle PE
  - this walrus build cannot encode multi-wait Drain/STT instructions:
    _fix_tail_drain_waits() rewrites multi-wait instructions into
    single-wait EventSemaphores
"""

import numpy as np

import concourse.bass as bass
import concourse.mybir as mybir
import concourse.tile as tile
from concourse.bass_utils import run_bass_kernel_spmd
from concourse.masks import make_identity

B, N, D = 8, 8192, 512


def _fix_tail_drain_waits(nc):
    """This image's walrus cannot encode more than one sem wait on several
    instruction kinds (Tile's end-of-kernel Drain, STT, ...). Move each
    wait of any multi-wait instruction onto its own EventSemaphore
    inserted right before it on the same engine — identical semantics
    (engine program order), always encodable."""
    for fn in nc.m.functions:
        for bb in fn.blocks:
            new = []
            for inst in bb.instructions:
                si = inst.sync_info
                if (
                    not isinstance(inst, mybir.InstEventSemaphore)
                    and si is not None
                    and si.on_wait
                    and len(si.on_wait) > 1
                ):
                    for k, w in enumerate(list(si.on_wait)):
                        ev = mybir.InstEventSemaphore(
                            name=f"{inst.name}-prewait{k}", ins=[], outs=[]
                        )
                        ev.engine = inst.engine
                        ev.sync_info = mybir.SyncInfo(on_wait=[w], on_update=[])
                        new.append(ev)
                    inst.sync_info = mybir.SyncInfo(
                        on_wait=[], on_update=list(si.on_update)
                    )
                new.append(inst)
            bb.instructions[:] = new


P = 128                    # SBUF partitions (rows per tile)
NT = N // P                # 64 row-tiles per core
GJ = 8                     # row-tiles per slot (3 x 2 MiB DMAs / slot)
NS = NT // GJ              # 8 slots
EPS = 1e-12
F32 = mybir.dt.float32
Mult = mybir.AluOpType.mult
Add = mybir.AluOpType.add
Sq = mybir.ActivationFunctionType.Square
Sqrt = mybir.ActivationFunctionType.Sqrt
H = D // 2


def build_nc():
    nc = bass.Bass(trn_type="TRN2")
    s_h = nc.declare_dram_parameter("s", [N, D], F32, isOutput=False)
    hrl_h = nc.declare_dram_parameter("h_rl", [N, D], F32, isOutput=False)
    hfk_h = nc.declare_dram_parameter("h_fk", [N, D], F32, isOutput=False)
    out_h = nc.declare_dram_parameter("out", [2, NT, P], F32, isOutput=True)

    # DRAM view: row r = (g*GJ + j)*P + p  ->  [g, p, j, d]
    def grouped(h):
        return h[:, :].rearrange("(g j p) d -> g p j d", j=GJ, p=P)

    s_g, hrl_g, hfk_g = grouped(s_h), grouped(hrl_h), grouped(hfk_h)

    with tile.TileContext(nc) as tc:
        with (
            tc.tile_pool(name="ins", bufs=2) as ins,
            tc.tile_pool(name="scrd", bufs=2) as scrd,
            tc.tile_pool(name="scra", bufs=2) as scra,
            tc.tile_pool(name="scrg", bufs=2) as scrg,
            tc.tile_pool(name="stats", bufs=1) as stats,
            tc.tile_pool(name="fin", bufs=1) as fin,
            tc.tile_pool(name="psum", bufs=1, space="PSUM") as psum,
        ):
            # per-row accumulators, column t = global tile index
            ss = stats.tile([P, NT], F32, tag="ss")      # sum s^2    (ACT)
            hh1 = stats.tile([P, NT], F32, tag="hh1")    # sum hrl^2  (ACT)
            hh2 = stats.tile([P, NT], F32, tag="hh2")    # sum hfk^2  (DVE)
            sp1 = stats.tile([P, NT], F32, tag="sp1")    # sum s*hrl  (DVE)
            sp2 = stats.tile([P, NT], F32, tag="sp2")    # sum s*hfk  (GPS+DVE)

            for g in range(NS):
                s_t = ins.tile([P, GJ, D], F32, tag="s")
                h1_t = ins.tile([P, GJ, D], F32, tag="h_rl")
                h2_t = ins.tile([P, GJ, D], F32, tag="h_fk")
                # h_fk first: its consumers (gpsimd mult chain) have the
                # longest post-DMA critical path
                nc.sync.dma_start(out=h2_t, in_=hfk_g[g])
                nc.sync.dma_start(out=s_t, in_=s_g[g])
                nc.sync.dma_start(out=h1_t, in_=hrl_g[g])

                scr_p = scrd.tile([P, D], F32, tag="scr_p")
                scr_q = scrd.tile([P, D], F32, tag="scr_q")
                scr_a = scra.tile([P, D], F32, tag="scr_a")
                scr_b = scra.tile([P, D], F32, tag="scr_b")
                p2 = scrg.tile([P, GJ, D], F32, tag="p2")
                p2h = scrg.tile([P, GJ, H], F32, tag="p2h")

                # gpsimd: p2 = s * h_fk, then pairwise halve
                nc.gpsimd.tensor_tensor(out=p2, in0=s_t, in1=h2_t, op=Mult)
                nc.gpsimd.tensor_tensor(
                    out=p2h, in0=p2[:, :, 0:H], in1=p2[:, :, H:D], op=Add)

                cols = slice(g * GJ, (g + 1) * GJ)
                for j in range(GJ):
                    t = g * GJ + j
                    tc1 = slice(t, t + 1)
                    # ACT: squares of s and h_rl, row-sum via accum_out
                    nc.scalar.activation(
                        out=scr_a, in_=s_t[:, j], func=Sq,
                        accum_out=ss[:, tc1])
                    nc.scalar.activation(
                        out=scr_b, in_=h1_t[:, j], func=Sq,
                        accum_out=hh1[:, tc1])
                    # DVE: s*h_rl dot and h_fk^2 norm via fused STT accum
                    nc.vector.scalar_tensor_tensor(
                        out=scr_p, in0=s_t[:, j], scalar=1.0, in1=h1_t[:, j],
                        op0=Mult, op1=Mult, accum_out=sp1[:, tc1])
                    nc.vector.scalar_tensor_tensor(
                        out=scr_q, in0=h2_t[:, j], scalar=1.0, in1=h2_t[:, j],
                        op0=Mult, op1=Mult, accum_out=hh2[:, tc1])
                # DVE: finish sp2 from the gpsimd-halved products
                nc.vector.tensor_reduce(
                    out=sp2[:, cols], in_=p2h,
                    axis=mybir.AxisListType.X, op=Add)

            # ---- finals on [P, NT] stats tiles; DVE except sqrt (ACT
            # Rsqrt/Reciprocal are banned for accuracy) ----
            ns_t = fin.tile([P, NT], F32, tag="ns_t")
            n1_t = fin.tile([P, NT], F32, tag="n1_t")
            n2_t = fin.tile([P, NT], F32, tag="n2_t")
            nc.scalar.activation(out=ns_t, in_=ss, func=Sqrt)
            nc.scalar.activation(out=n1_t, in_=hh1, func=Sqrt)
            nc.scalar.activation(out=n2_t, in_=hh2, func=Sqrt)
            nc.vector.tensor_scalar_max(ns_t, ns_t, EPS)
            nc.vector.tensor_scalar_max(n1_t, n1_t, EPS)
            nc.vector.tensor_scalar_max(n2_t, n2_t, EPS)
            den1 = fin.tile([P, NT], F32, tag="den1")
            den2 = fin.tile([P, NT], F32, tag="den2")
            nc.vector.tensor_tensor(out=den1, in0=ns_t, in1=n1_t, op=Mult)
            nc.vector.tensor_tensor(out=den2, in0=ns_t, in1=n2_t, op=Mult)
            nc.vector.reciprocal(den1, den1)
            nc.vector.reciprocal(den2, den2)
            o1 = fin.tile([P, NT], F32, tag="o1")
            o2 = fin.tile([P, NT], F32, tag="o2")
            nc.vector.tensor_tensor(out=o1, in0=sp1, in1=den1, op=Mult)
            nc.vector.tensor_tensor(out=o2, in0=sp2, in1=den2, op=Mult)

            # transpose [P, NT] -> [NT, P] on the (idle) tensor engine
            ident = fin.tile([P, P], F32, tag="ident")
            make_identity(nc, ident)
            po1 = psum.tile([NT, P], F32, tag="po1")
            po2 = psum.tile([NT, P], F32, tag="po2")
            nc.tensor.transpose(po1, o1, ident)
            nc.tensor.transpose(po2, o2, ident)
            o1t = fin.tile([NT, P], F32, tag="o1t")
            o2t = fin.tile([NT, P], F32, tag="o2t")
            nc.scalar.copy(o1t, po1)
            nc.scalar.copy(o2t, po2)
            nc.sync.dma_start(out=out_h[0], in_=o1t)
            nc.sync.dma_start(out=out_h[1], in_=o2t)

    _fix_tail_drain_waits(nc)
    return nc


_NC_CACHE = None


def kernel(s, h_rl, h_fk, trace=False):
    global _NC_CACHE
    s = np.ascontiguousarray(np.asarray(s, dtype=np.float32))
    h_rl = np.ascontiguousarray(np.asarray(h_rl, dtype=np.float32))
    h_fk = np.ascontiguousarray(np.asarray(h_fk, dtype=np.float32))
    assert s.shape == (B, N, D), s.shape

    if _NC_CACHE is None:
        _NC_CACHE = build_nc()
    nc = _NC_CACHE

    in_maps = [
        {"s": s[b], "h_rl": h_rl[b], "h_fk": h_fk[b]} for b in range(B)
    ]
    res = run_bass_kernel_spmd(nc, in_maps, core_ids=list(range(B)), trace=trace)
    out = np.empty((B, 2 * N), dtype=np.float32)
    for b in range(B):
        o = res.results[b]["out"].reshape(2, N)
        out[b, :N] = o[0]
        out[b, N:] = o[1]
    if trace:
        return out, res
    return out


# revision 5
# speedup vs baseline: 1.5512x; 1.5512x over previous
"""Trainium2 Bass kernel: row-wise cosine similarity discriminator.

Computes, for full inputs s, h_rl, h_fk of shape [B=8, N=8192, D=512] f32:
    out = concat(rowdot(l2n(s), l2n(h_rl)), rowdot(l2n(s), l2n(h_fk)), axis=1)
with l2n(x) = x / max(||x||_2, 1e-12), giving out shape [8, 16384] f32.

Sharding: pure data parallel over batch B — core b processes batch b.

Per-core kernel strategy (memory-bound: 48 MiB input / core, HBM floor
~141 us at ~358 GB/s/core). All engine work is kept under that floor by
fusing every row-reduction into the instruction that produces it:
  - DVE  scalar_tensor_tensor(mult, accum_out): s*h_rl and s*h_fk dots
    plus h_fk^2 norm, one [128, 512] tile-job each (~0.7 us)
  - ACT  activation(Square, accum_out): s^2 and h_rl^2 norms (~0.9 us
    per tile-job incl. the 186 ns ACTIVATION_READ_ACCUMULATOR)
  - GPSIMD does NOTHING: STT is a TensorScalarPtr-family op, which
    grabs the DVE<->GpSimd shared SBUF port pair as an exclusive lock;
    any long-running gpsimd op blocks every STT for its remainder
    (measured: 0.69 us -> 1.7 us avg with gpsimd mults in flight)
  - data streamed in slots of [128, <=8, 512] x 3 tensors (HWDGE DMAs),
    double-buffered, small first/last slots for ramp and tail; finals
    on [128, 64] stats tiles, output transposed on the idle PE
  - this walrus build cannot encode multi-wait Drain/STT instructions:
    _fix_tail_drain_waits() rewrites multi-wait instructions into
    single-wait EventSemaphores
"""

import numpy as np

import concourse.bass as bass
import concourse.mybir as mybir
import concourse.tile as tile
from concourse.bass_utils import run_bass_kernel_spmd
from concourse.masks import make_identity

B, N, D = 8, 8192, 512


def _fix_tail_drain_waits(nc):
    """This image's walrus cannot encode more than one sem wait on several
    instruction kinds (Tile's end-of-kernel Drain, STT, ...). Move each
    wait of any multi-wait instruction onto its own EventSemaphore
    inserted right before it on the same engine — identical semantics
    (engine program order), always encodable."""
    for fn in nc.m.functions:
        for bb in fn.blocks:
            new = []
            for inst in bb.instructions:
                si = inst.sync_info
                if (
                    not isinstance(inst, mybir.InstEventSemaphore)
                    and si is not None
                    and si.on_wait
                    and len(si.on_wait) > 1
                ):
                    for k, w in enumerate(list(si.on_wait)):
                        ev = mybir.InstEventSemaphore(
                            name=f"{inst.name}-prewait{k}", ins=[], outs=[]
                        )
                        ev.engine = inst.engine
                        ev.sync_info = mybir.SyncInfo(on_wait=[w], on_update=[])
                        new.append(ev)
                    inst.sync_info = mybir.SyncInfo(
                        on_wait=[], on_update=list(si.on_update)
                    )
                new.append(inst)
            bb.instructions[:] = new


P = 128                    # SBUF partitions (rows per tile)
NT = N // P                # 64 row-tiles per core
GJ = 8                     # max row-tiles per slot (3 x 2 MiB DMAs)
SLOTS = [4, 4, 8, 8, 8, 8, 8, 8, 4, 4]   # sums to NT
EPS = 1e-12
F32 = mybir.dt.float32
Mult = mybir.AluOpType.mult
Add = mybir.AluOpType.add
Sq = mybir.ActivationFunctionType.Square
Sqrt = mybir.ActivationFunctionType.Sqrt
H = D // 2


def build_nc():
    nc = bass.Bass(trn_type="TRN2")
    s_h = nc.declare_dram_parameter("s", [N, D], F32, isOutput=False)
    hrl_h = nc.declare_dram_parameter("h_rl", [N, D], F32, isOutput=False)
    hfk_h = nc.declare_dram_parameter("h_fk", [N, D], F32, isOutput=False)
    out_h = nc.declare_dram_parameter("out", [2, NT, P], F32, isOutput=True)

    # DRAM view: row r = (t0 + j)*P + p  ->  [p, j, d] for a slot of
    # `w` row-tiles starting at global tile t0
    def slot_ap(h, t0, w):
        return h[t0 * P : (t0 + w) * P, :].rearrange(
            "(j p) d -> p j d", j=w, p=P)

    with tile.TileContext(nc) as tc:
        with (
            tc.tile_pool(name="ins", bufs=2) as ins,
            tc.tile_pool(name="scrd", bufs=2) as scrd,
            tc.tile_pool(name="scra", bufs=2) as scra,
            tc.tile_pool(name="stats", bufs=1) as stats,
            tc.tile_pool(name="fin", bufs=1) as fin,
            tc.tile_pool(name="psum", bufs=1, space="PSUM") as psum,
        ):
            # per-row accumulators, column t = global tile index
            ss = stats.tile([P, NT], F32, tag="ss")      # sum s^2    (ACT)
            hh1 = stats.tile([P, NT], F32, tag="hh1")    # sum hrl^2  (ACT)
            hh2 = stats.tile([P, NT], F32, tag="hh2")    # sum hfk^2  (DVE)
            sp1 = stats.tile([P, NT], F32, tag="sp1")    # sum s*hrl  (DVE)
            sp2 = stats.tile([P, NT], F32, tag="sp2")    # sum s*hfk  (DVE)

            t0 = 0
            for w in SLOTS:
                s_t = ins.tile([P, GJ, D], F32, name="s_t", tag="s")
                h1_t = ins.tile([P, GJ, D], F32, name="h1_t", tag="h_rl")
                h2_t = ins.tile([P, GJ, D], F32, name="h2_t", tag="h_fk")
                s_t, h1_t, h2_t = s_t[:, :w], h1_t[:, :w], h2_t[:, :w]
                nc.sync.dma_start(out=s_t, in_=slot_ap(s_h, t0, w))
                nc.sync.dma_start(out=h1_t, in_=slot_ap(hrl_h, t0, w))
                nc.sync.dma_start(out=h2_t, in_=slot_ap(hfk_h, t0, w))

                scr_p = scrd.tile([P, D], F32, tag="scr_p")
                scr_a = scra.tile([P, D], F32, tag="scr_a")

                for j in range(w):
                    t = t0 + j
                    tc1 = slice(t, t + 1)
                    # ACT: squares of s and h_rl, row-sum via accum_out
                    nc.scalar.activation(
                        out=scr_a, in_=s_t[:, j], func=Sq,
                        accum_out=ss[:, tc1])
                    nc.scalar.activation(
                        out=scr_a, in_=h1_t[:, j], func=Sq,
                        accum_out=hh1[:, tc1])
                    # DVE: both dots and the h_fk^2 norm via fused STT accum
                    nc.vector.scalar_tensor_tensor(
                        out=scr_p, in0=s_t[:, j], scalar=1.0, in1=h1_t[:, j],
                        op0=Mult, op1=Mult, accum_out=sp1[:, tc1])
                    nc.vector.scalar_tensor_tensor(
                        out=scr_p, in0=s_t[:, j], scalar=1.0, in1=h2_t[:, j],
                        op0=Mult, op1=Mult, accum_out=sp2[:, tc1])
                    nc.vector.scalar_tensor_tensor(
                        out=scr_p, in0=h2_t[:, j], scalar=1.0, in1=h2_t[:, j],
                        op0=Mult, op1=Mult, accum_out=hh2[:, tc1])
                t0 += w

            # ---- finals on [P, NT] stats tiles; DVE except sqrt (ACT
            # Rsqrt/Reciprocal are banned for accuracy) ----
            ns_t = fin.tile([P, NT], F32, tag="ns_t")
            n1_t = fin.tile([P, NT], F32, tag="n1_t")
            n2_t = fin.tile([P, NT], F32, tag="n2_t")
            nc.scalar.activation(out=ns_t, in_=ss, func=Sqrt)
            nc.scalar.activation(out=n1_t, in_=hh1, func=Sqrt)
            nc.scalar.activation(out=n2_t, in_=hh2, func=Sqrt)
            nc.vector.tensor_scalar_max(ns_t, ns_t, EPS)
            nc.vector.tensor_scalar_max(n1_t, n1_t, EPS)
            nc.vector.tensor_scalar_max(n2_t, n2_t, EPS)
            den1 = fin.tile([P, NT], F32, tag="den1")
            den2 = fin.tile([P, NT], F32, tag="den2")
            nc.vector.tensor_tensor(out=den1, in0=ns_t, in1=n1_t, op=Mult)
            nc.vector.tensor_tensor(out=den2, in0=ns_t, in1=n2_t, op=Mult)
            nc.vector.reciprocal(den1, den1)
            nc.vector.reciprocal(den2, den2)
            o1 = fin.tile([P, NT], F32, tag="o1")
            o2 = fin.tile([P, NT], F32, tag="o2")
            nc.vector.tensor_tensor(out=o1, in0=sp1, in1=den1, op=Mult)
            nc.vector.tensor_tensor(out=o2, in0=sp2, in1=den2, op=Mult)

            # transpose [P, NT] -> [NT, P] on the (idle) tensor engine
            ident = fin.tile([P, P], F32, tag="ident")
            make_identity(nc, ident)
            po1 = psum.tile([NT, P], F32, tag="po1")
            po2 = psum.tile([NT, P], F32, tag="po2")
            nc.tensor.transpose(po1, o1, ident)
            nc.tensor.transpose(po2, o2, ident)
            o1t = fin.tile([NT, P], F32, tag="o1t")
            o2t = fin.tile([NT, P], F32, tag="o2t")
            nc.scalar.copy(o1t, po1)
            nc.scalar.copy(o2t, po2)
            nc.sync.dma_start(out=out_h[0], in_=o1t)
            nc.sync.dma_start(out=out_h[1], in_=o2t)

    _fix_tail_drain_waits(nc)
    return nc


_NC_CACHE = None


def kernel(s, h_rl, h_fk, trace=False):
    global _NC_CACHE
    s = np.ascontiguousarray(np.asarray(s, dtype=np.float32))
    h_rl = np.ascontiguousarray(np.asarray(h_rl, dtype=np.float32))
    h_fk = np.ascontiguousarray(np.asarray(h_fk, dtype=np.float32))
    assert s.shape == (B, N, D), s.shape

    if _NC_CACHE is None:
        _NC_CACHE = build_nc()
    nc = _NC_CACHE

    in_maps = [
        {"s": s[b], "h_rl": h_rl[b], "h_fk": h_fk[b]} for b in range(B)
    ]
    res = run_bass_kernel_spmd(nc, in_maps, core_ids=list(range(B)), trace=trace)
    out = np.empty((B, 2 * N), dtype=np.float32)
    for b in range(B):
        o = res.results[b]["out"].reshape(2, N)
        out[b, :N] = o[0]
        out[b, N:] = o[1]
    if trace:
        return out, res
    return out
